# revision 14
# baseline (speedup 1.0000x reference)
import sys

for _p in ("/opt/trn_rl_repo", "/root/.axon_site/_ro/trn_rl_repo"):
    if _p not in sys.path:
        sys.path.append(_p)

import numpy as np

# Problem: B=8 batches of cross-attention-like softmax matmul, one batch per core.
#   S[e,t] = sum_d enc[e,d] * dec[t,d]
#   A = softmax(S, axis=t)
#   C[t,d] = sum_e A[e,t] * enc[e,d]
B, S, D = 8, 2048, 1024
P = 128
EB = S // P   # 16 e-blocks
TB = S // P   # 16 t-blocks
DC = D // P   # 8 d-chunks (contraction for scores)
TC = S // 512 # 4 t-chunks of 512 (matmul free-dim limit)

_NC_CACHE = None


def _build():
    import concourse.bacc as bacc
    import concourse.tile as tile
    from concourse import mybir
    from concourse.masks import make_identity

    F32 = mybir.dt.float32
    F16 = mybir.dt.float16

    nc = bacc.Bacc("TRN2", target_bir_lowering=False, debug=False, num_devices=B)
    enc = nc.declare_dram_parameter("enc_outputs", [S, D], F32, isOutput=False)
    dec = nc.declare_dram_parameter("dec_outputs", [S, D], F32, isOutput=False)
    out = nc.declare_dram_parameter("out", [S, D], F32, isOutput=True)

    with tile.TileContext(nc) as tc:
        with (
            tc.tile_pool(name="const", bufs=1) as const_pool,
            tc.tile_pool(name="bigT", bufs=1) as bigT_pool,
            tc.tile_pool(name="encn", bufs=1) as encn_pool,
            tc.tile_pool(name="decn", bufs=4) as decn_pool,
            tc.tile_pool(name="pmat", bufs=1) as p_pool,
            tc.tile_pool(name="stats", bufs=4) as stats_pool,
            tc.tile_pool(name="ostage", bufs=3) as out_pool,
        ):
            ident = const_pool.tile([P, P], F16, name="ident")
            make_identity(nc, ident)

            # d-major transposed operands, one big tile each:
            # encTbig[:, d*S + e*P + j] = enc[e*P + j, d*P + dd]  (dd = partition)
            encTbig = bigT_pool.tile([P, DC * S], F16, name="encTbig")
            decTbig = bigT_pool.tile([P, DC * S], F16, name="decTbig")
            encn = [encn_pool.tile([P, D], F16, name=f"encn{e}") for e in range(EB)]
            pmat = [p_pool.tile([P, S], F16, name=f"p{e}") for e in range(EB)]

            # Cast loads (f32 DRAM -> fp16 SBUF via SWDGE cast-DMA), ordered so
            # eb0's first score matmuls can start as early as possible:
            # dec blocks 0-7, enc block 0, dec blocks 8-15, enc blocks 1-15.
            dec_tiles = [None] * TB
            enc_loaded = [False] * EB

            def cast_dec(t):
                dtile = decn_pool.tile([P, D], F16, name="decn", tag="decn")
                if t == 0:
                    for r in range(4):
                        nc.gpsimd.dma_start(
                            out=dtile[r * 32 : (r + 1) * 32, :],
                            in_=dec[r * 32 : (r + 1) * 32, :],
                        )
                else:
                    nc.gpsimd.dma_start(out=dtile[:], in_=dec[t * P : (t + 1) * P, :])
                dec_tiles[t] = dtile

            def cast_enc(e):
                nc.gpsimd.dma_start(out=encn[e][:], in_=enc[e * P : (e + 1) * P, :])
                enc_loaded[e] = True

            for t in range(8):
                cast_dec(t)
            cast_enc(0)
            for t in range(8, TB):
                cast_dec(t)
            for e in range(1, EB):
                cast_enc(e)

            gi = [0]

            with tc.tile_pool(name="psum_s", bufs=1, space="PSUM") as psum_s:
                # tags: "sps" = half-score tiles [128,1024] f32 (2 banks) x3 bufs,
                #       "tp"  = transpose staging [128,1024] f16 (1 bank) x2 bufs.
                def t_group(src, tgt, blk):
                    tp = psum_s.tile(
                        [P, D], F16, tag="tp", bufs=2, name=f"tp{gi[0]}"
                    )
                    for d in range(DC):
                        nc.tensor.transpose(
                            tp[:, d * P : (d + 1) * P],
                            src[:, d * P : (d + 1) * P],
                            ident,
                        )
                    src3 = tp[:].rearrange("p (d s) -> p d s", d=DC)
                    dst3 = tgt[:].rearrange("p (d s) -> p d s", d=DC)[
                        :, :, blk * P : (blk + 1) * P
                    ]
                    if gi[0] % 3 == 2:
                        nc.scalar.copy(out=dst3, in_=src3)
                    else:
                        nc.vector.tensor_copy(out=dst3, in_=src3)
                    gi[0] += 1

                def eb_half(e, half, sps):
                    # half 0 -> t-chunks 0,1 ; half 1 -> t-chunks 2,3
                    for tt in range(2):
                        t = half * 2 + tt
                        for d in range(DC):
                            nc.tensor.matmul(
                                sps[:, tt * 512 : (tt + 1) * 512],
                                lhsT=encTbig[:, d * S + e * P : d * S + (e + 1) * P],
                                rhs=decTbig[:, d * S + t * 512 : d * S + (t + 1) * 512],
                                start=(d == 0),
                                stop=(d == DC - 1),
                            )

                def softmax_eb(e, spsA, spsB):
                    pmax = stats_pool.tile([P, TC], F32, name="pmax")
                    for half, sps in ((0, spsA), (1, spsB)):
                        for tt in range(2):
                            nc.vector.reduce_max(
                                out=pmax[:, half * 2 + tt : half * 2 + tt + 1],
                                in_=sps[:, tt * 512 : (tt + 1) * 512],
                                axis=mybir.AxisListType.X,
                            )
                    negmax = stats_pool.tile([P, 1], F32, name="negmax")
                    nc.vector.reduce_max(
                        out=negmax, in_=pmax[:], axis=mybir.AxisListType.X, negate=True
                    )
                    za = stats_pool.tile([P, 1], F32, name="za")
                    zb = stats_pool.tile([P, 1], F32, name="zb")
                    nc.scalar.activation(
                        out=pmat[e][:, 0:D],
                        in_=spsA[:],
                        func=mybir.ActivationFunctionType.Exp,
                        bias=negmax,
                        scale=1.0,
                        accum_out=za,
                    )
                    nc.scalar.activation(
                        out=pmat[e][:, D : 2 * D],
                        in_=spsB[:],
                        func=mybir.ActivationFunctionType.Exp,
                        bias=negmax,
                        scale=1.0,
                        accum_out=zb,
                    )
                    z = stats_pool.tile([P, 1], F32, name="zz")
                    nc.vector.tensor_add(out=z, in0=za, in1=zb)
                    zinv = stats_pool.tile([P, 1], F32, name="zinv")
                    nc.vector.reciprocal(zinv, z)
                    nc.vector.tensor_scalar_mul(encn[e][:], encn[e][:], zinv)

                # eb0 interleaved with the transpose/cast stream
                for blk in range(8):
                    t_group(dec_tiles[blk], decTbig, blk)
                t_group(encn[0], encTbig, 0)
                spsA0 = psum_s.tile([P, D], F32, tag="sps", bufs=3, name="spsA0")
                eb_half(0, 0, spsA0)
                for blk in range(8, TB):
                    t_group(dec_tiles[blk], decTbig, blk)
                spsB0 = psum_s.tile([P, D], F32, tag="sps", bufs=3, name="spsB0")
                eb_half(0, 1, spsB0)
                softmax_eb(0, spsA0, spsB0)

                for e in range(1, EB):
                    t_group(encn[e], encTbig, e)
                    spsA = psum_s.tile([P, D], F32, tag="sps", bufs=3, name=f"spsA{e}")
                    eb_half(e, 0, spsA)
                    spsB = psum_s.tile([P, D], F32, tag="sps", bufs=3, name=f"spsB{e}")
                    eb_half(e, 1, spsB)
                    softmax_eb(e, spsA, spsB)

                # Phase C: context C[t,:] = sum_e P[e,t] * encZ[e,:]
                for t in range(TB):
                    c_ps = psum_s.tile([P, D], F32, tag="sps", bufs=3, name=f"c_ps{t}")
                    for e in range(EB):
                        for hf in range(2):
                            nc.tensor.matmul(
                                c_ps[:, hf * 512 : (hf + 1) * 512],
                                lhsT=pmat[e][:, t * P : (t + 1) * P],
                                rhs=encn[e][:, hf * 512 : (hf + 1) * 512],
                                start=(e == 0),
                                stop=(e == EB - 1),
                            )
                    o_t = out_pool.tile([P, D], F32, name="o_t")
                    for hf in range(2):
                        nc.any.tensor_copy(
                            out=o_t[:, hf * 512 : (hf + 1) * 512],
                            in_=c_ps[:, hf * 512 : (hf + 1) * 512],
                        )
                        nc.scalar.dma_start(
                            out=out[t * P : (t + 1) * P, hf * 512 : (hf + 1) * 512],
                            in_=o_t[:, hf * 512 : (hf + 1) * 512],
                        )

    nc.compile()
    return nc


def _get_nc():
    global _NC_CACHE
    if _NC_CACHE is None:
        _NC_CACHE = _build()
    return _NC_CACHE


def kernel(enc_outputs, dec_outputs, _want_results=False, **_ignored):
    from concourse.bass_utils import run_bass_kernel_spmd

    nc = _get_nc()
    enc_outputs = np.asarray(enc_outputs, dtype=np.float32)
    dec_outputs = np.asarray(dec_outputs, dtype=np.float32)
    in_maps = [
        {
            "enc_outputs": np.ascontiguousarray(enc_outputs[b]),
            "dec_outputs": np.ascontiguousarray(dec_outputs[b]),
        }
        for b in range(B)
    ]
    res = run_bass_kernel_spmd(nc, in_maps, core_ids=list(range(B)))
    out = np.stack([res.results[b]["out"] for b in range(B)], axis=0)
    if _want_results:
        return out, res
    return out
